# revision 20
# baseline (speedup 1.0000x reference)
"""Multi-head causal attention (B=4,S=1024,D=768,H=12,Dh=64) on 8 trn2 cores.

Sharding: core c handles batch b=c//2 and the 6 heads hs=(c%2)*6 .. hs+6
(head-axis tensor parallel x batch parallel; 8 cores = 4 batches x 2 head-halves).

Per-core on-chip dataflow (bf16 matmul operands, fp32 PSUM accumulation):
  xT [768,1024] (host-pretransposed bf16), W{q,k,v} stacked [768, 384] bf16
  qT/kT = W-chunk.T(lhsT) @ xT    -> [64,1024] per head (transposed layout)
  v     = xT-chunk.T @ Wv          -> [1024, 6*65] per t-chunk (65th col = ones)
  scoresT[t,s] tiles = kT-chunk(lhsT) x qT(rhs); fully-causal tiles skipped,
  diag tiles masked by accumulating identity @ (-30000 strict-lower-tri) in PSUM
  exp via ScalarE Exp(scale=1/8) straight from PSUM into a flat bf16 SBUF buffer
  ctxT_aug[65, s] = sum_j v_aug_j(lhsT) @ expT_j  (row 64 = softmax denominator)
  y_aug[h, 0:65, s] DMA'd out fp32; host divides by denominators + transposes.
"""

import threading
from contextlib import ExitStack

import ml_dtypes
import numpy as np

import concourse.bass as bass
import concourse.tile as tile
from concourse import bacc, mybir
from concourse.bass_utils import run_bass_kernel_spmd

B, S, D, H, DH = 4, 1024, 768, 12, 64
NCORES = 8
HL = H // 2          # 6 local heads per core
KC = D // 128        # 6 contraction chunks
NPAIR = HL // 2      # head pairs for qk projection
F32 = mybir.dt.float32
BF16 = mybir.dt.bfloat16
MASK_VAL = -30000.0


def _attn_groups():
    """Chunk table for one head's scoresT, packed into [128,1024] PSUM groups.

    A chunk (j, c) is the scoresT tile for t-chunk j (rows j*128..j*128+128)
    and s-range [s0, s0+w) inside output half c (s in [512c, 512c+512)).
    Only causal-relevant chunks exist. `diag` chunks need the triangular mask
    added to their first 128 columns. `ps_off` is the column offset inside the
    group's PSUM tile (each chunk stays inside one 512-col PSUM bank);
    `off` is the offset in the per-head flat exp buffer.
    """
    def chunk(j, c, ps_off):
        s0 = max(512 * c, 128 * j)
        w = 512 * (c + 1) - s0
        return dict(j=j, c=c, s0=s0, w=w, diag=(s0 == 128 * j), ps_off=ps_off)

    groups = [
        [chunk(0, 1, 0), chunk(0, 0, 512)],
        [chunk(1, 1, 0), chunk(1, 0, 512), chunk(7, 1, 896)],
        [chunk(2, 1, 0), chunk(2, 0, 512), chunk(6, 1, 768)],
        [chunk(3, 1, 0), chunk(3, 0, 512), chunk(5, 1, 640)],
        [chunk(4, 1, 0)],
    ]
    base = 0
    for g in groups:
        for ch in g:
            ch["off"] = base + ch["ps_off"]
        g_w = max(ch["ps_off"] + ch["w"] for ch in g)
        base += g_w
    total = base  # 4608
    return groups, total


def _emit_kernel(ctx: ExitStack, tc: tile.TileContext, xT, wq, wk, wv, im, y):
    nc = tc.nc
    groups, exp_cols = _attn_groups()

    # identity + causal mask arrive as a tiny host input (generating them
    # on GpSimd costs ~6us and delays the PE warm-up)
    const = ctx.enter_context(tc.tile_pool(name="const", bufs=1))
    im_sb = const.tile([128, 2, 128], BF16)
    nc.sync.dma_start(out=im_sb, in_=im[:, :, :])
    ident = im_sb[:, 0, :]
    mask = im_sb[:, 1, :]

    qk_pool = ctx.enter_context(tc.tile_pool(name="qk", bufs=1))
    qT = qk_pool.tile([128, NPAIR, S], BF16)  # partitions: (h%2)*64+e, pair, s
    kT = qk_pool.tile([128, NPAIR, S], BF16)
    v_sb = qk_pool.tile([128, 8, HL * (DH + 1)], BF16)  # [t_rel, t_chunk, h*65+x]

    # pools (PSUM budget: pj 2 banks + sg 2x2 + cx 2 = 8)
    xtw = ctx.enter_context(tc.tile_pool(name="xtw", bufs=1))
    pj = ctx.enter_context(tc.tile_pool(name="pj", bufs=1, space="PSUM"))
    sg = ctx.enter_context(tc.tile_pool(name="sg", bufs=2, space="PSUM"))
    cx = ctx.enter_context(tc.tile_pool(name="cx", bufs=2, space="PSUM"))
    ex = ctx.enter_context(tc.tile_pool(name="ex", bufs=4))
    yst = ctx.enter_context(tc.tile_pool(name="yst", bufs=3))

    # PE warm-up: ~3.5us of dummy matmuls into a scratch PSUM bank so the HAM
    # clock gate opens (K=8/8, 2.4 GHz) before the real matmuls arrive.
    warm = pj.tile([128, 128], F32, tag="pjq0", name="warm")
    for i in range(44):
        nc.tensor.matmul(out=warm, lhsT=ident, rhs=mask,
                         start=(i == 0), stop=(i == 43))

    xt = xtw.tile([128, KC, S], BF16)
    w_q = xtw.tile([128, KC, HL * DH], BF16)
    w_k = xtw.tile([128, KC, HL * DH], BF16)
    w_v = xtw.tile([128, KC, HL * DH], BF16)
    # per-chunk loads spread over four DMA queues so chunk 0 lands fast and
    # the four streams share HBM bandwidth
    for kc in range(KC):
        nc.sync.dma_start(out=xt[:, kc, :], in_=xT[kc * 128:(kc + 1) * 128, :])
        nc.scalar.dma_start(out=w_q[:, kc, :], in_=wq[kc * 128:(kc + 1) * 128, :])
        nc.scalar.dma_start(out=w_k[:, kc, :], in_=wk[kc * 128:(kc + 1) * 128, :])
        nc.gpsimd.dma_start(out=w_v[:, kc, :], in_=wv[kc * 128:(kc + 1) * 128, :])

    def proj_qk(pp):
        """q/k projection for head pair pp; kc-outer so chunk-0 DMAs suffice."""
        for w_all, dst in ((w_q, qT), (w_k, kT)):
            ps0 = pj.tile([128, 512], F32, tag="pjq0", name=f"psq0_{pp}")
            ps1 = pj.tile([128, 512], F32, tag="pjq1", name=f"psq1_{pp}")
            for kc in range(KC):
                for i, ps in enumerate((ps0, ps1)):
                    nc.tensor.matmul(
                        out=ps,
                        lhsT=w_all[:, kc, pp * 128:(pp + 1) * 128],
                        rhs=xt[:, kc, i * 512:(i + 1) * 512],
                        start=(kc == 0), stop=(kc == KC - 1),
                    )
            for i, ps in enumerate((ps0, ps1)):
                nc.vector.tensor_copy(
                    out=dst[:, pp, i * 512:(i + 1) * 512], in_=ps)

    def proj_v():
        for j in range(8):
            psv = pj.tile([128, HL * DH], F32, tag=f"pjq{j % 2}", name=f"psv{j}")
            for kc in range(KC):
                nc.tensor.matmul(
                    out=psv,
                    lhsT=xt[:, kc, j * 128:(j + 1) * 128],
                    rhs=w_v[:, kc, :],
                    start=(kc == 0), stop=(kc == KC - 1),
                )
            v_dst = v_sb[:, j, :].rearrange("p (h x) -> p h x", h=HL)
            nc.vector.tensor_copy(
                out=v_dst[:, :, 0:DH],
                in_=psv.rearrange("p (h e) -> p h e", h=HL),
            )
            nc.vector.memset(v_dst[:, :, DH:DH + 1], 1.0)

    def bank_ops(g, bank):
        """Matmul/mask op list for one PSUM bank of a scores group."""
        ops = [("mm", ch) for ch in g if ch["ps_off"] // 512 == bank]
        ops += [("mask", ch) for ch in g
                if ch["diag"] and ch["ps_off"] // 512 == bank]
        return ops

    def scores_pair(hp, exp_ts):
        """ScoresT + exp for heads 2hp (PE rows 0-63) and 2hp+1 (rows 64-127).

        The two heads' matmuls alternate in the stream; base_partition 0/64
        on lhsT auto-derives distinct PE row groups, so the K=64 matmuls run
        concurrently in the array.
        """
        for g in groups:
            g_w = max(ch["ps_off"] + ch["w"] for ch in g)
            ps_ab = [sg.tile([128, 1024], F32, tag="sg", name=f"sg{hp}{i}")
                     for i in range(2)]
            for bank in (0, 1):
                per_head = []
                for a in (0, 1):
                    half = a * 64
                    qT_h = qT[half:half + 64, hp, :]
                    kT_h = kT[half:half + 64, hp, :]
                    ops = bank_ops(g, bank)
                    per_head.append((qT_h, kT_h, ps_ab[a], ops))
                n_ops = len(per_head[0][3])
                for i in range(n_ops):
                    first, last = (i == 0), (i == n_ops - 1)
                    for qT_h, kT_h, ps, ops in per_head:
                        kind, ch = ops[i]
                        if kind == "mm":
                            nc.tensor.matmul(
                                out=ps[:, ch["ps_off"]:ch["ps_off"] + ch["w"]],
                                lhsT=kT_h[:, ch["j"] * 128:(ch["j"] + 1) * 128],
                                rhs=qT_h[:, ch["s0"]:ch["s0"] + ch["w"]],
                                start=first, stop=last,
                            )
                        else:
                            nc.tensor.matmul(
                                out=ps[:, ch["ps_off"]:ch["ps_off"] + 128],
                                lhsT=ident, rhs=mask,
                                start=first, stop=last,
                            )
            for a in (0, 1):
                nc.scalar.activation(
                    out=exp_ts[a][:, g[0]["off"]:g[0]["off"] + g_w],
                    in_=ps_ab[a][:, 0:g_w],
                    func=mybir.ActivationFunctionType.Exp,
                    scale=1.0 / np.sqrt(DH),
                )

    chunks = [ch for g in groups for ch in g]

    def ctx_head(h, exp_t):
        for c in (0, 1):
            cc = sorted((ch for ch in chunks if ch["c"] == c), key=lambda t: t["j"])
            pc = cx.tile([DH + 1, 512], F32, tag="cx", name=f"pc{h}{c}")
            for idx, ch in enumerate(cc):
                nc.tensor.matmul(
                    out=pc[:, ch["s0"] - 512 * c: ch["s0"] - 512 * c + ch["w"]],
                    lhsT=v_sb[:, ch["j"], :].rearrange(
                        "p (hh x) -> p hh x", hh=HL)[:, h, :],
                    rhs=exp_t[:, ch["off"]:ch["off"] + ch["w"]],
                    start=(idx == 0), stop=(idx == len(cc) - 1),
                )
            yt = yst.tile([DH + 1, 512], F32, tag="yst", name=f"yt{h}{c}")
            nc.vector.tensor_copy(out=yt, in_=pc)
            nc.sync.dma_start(out=y[h, :, c * 512:(c + 1) * 512], in_=yt)

    # schedule: proj0 -> scores01 -> v -> proj1 -> ctx01 -> scores23 -> ...
    proj_qk(0)
    exp_prev = None
    for hp in range(NPAIR):
        exp_ts = [ex.tile([128, exp_cols], BF16, tag="exp", name=f"exp{hp}{a}")
                  for a in (0, 1)]
        scores_pair(hp, exp_ts)
        if hp == 0:
            proj_v()
        if hp + 1 < NPAIR:
            proj_qk(hp + 1)
        if exp_prev is not None:
            ctx_head(2 * (hp - 1), exp_prev[0])
            ctx_head(2 * (hp - 1) + 1, exp_prev[1])
        exp_prev = exp_ts
    ctx_head(2 * (NPAIR - 1), exp_prev[0])
    ctx_head(2 * (NPAIR - 1) + 1, exp_prev[1])


_PROGRAM = None
_PROGRAM_LOCK = threading.Lock()


def _get_program() -> bass.Bass:
    global _PROGRAM
    with _PROGRAM_LOCK:
        if _PROGRAM is None:
            nc = bacc.Bacc(None, target_bir_lowering=False)
            xT = nc.declare_dram_parameter("xT", [D, S], BF16, isOutput=False)
            wq = nc.declare_dram_parameter("wq", [D, HL * DH], BF16, isOutput=False)
            wk = nc.declare_dram_parameter("wk", [D, HL * DH], BF16, isOutput=False)
            wv = nc.declare_dram_parameter("wv", [D, HL * DH], BF16, isOutput=False)
            im = nc.declare_dram_parameter("im", [128, 2, 128], BF16, isOutput=False)
            y = nc.declare_dram_parameter("y_aug", [HL, DH + 1, S], F32, isOutput=True)
            with tile.TileContext(nc) as tc, ExitStack() as ctx:
                _emit_kernel(ctx, tc, xT, wq, wk, wv, im, y)
            nc.finalize()  # runs Bacc passes (reg alloc, wait splitting)
            _PROGRAM = nc
    return _PROGRAM


def make_in_maps(x, Wq, Wk, Wv):
    """Per-core input dicts: batch b=core//2, heads (core%2)*6..+6."""
    bf = ml_dtypes.bfloat16
    im = np.zeros((128, 2, 128), np.float32)
    im[:, 0, :] = np.eye(128)
    t = np.arange(128)
    im[:, 1, :] = np.where(t[None, :] >= t[:, None], 0.0, MASK_VAL)
    im = im.astype(bf)
    in_maps = []
    for core in range(NCORES):
        b, hs = core // 2, (core % 2) * HL
        xTc = np.ascontiguousarray(np.asarray(x[b]).T.astype(bf))
        maps = {"xT": xTc, "im": im}
        for name, W in (("wq", Wq), ("wk", Wk), ("wv", Wv)):
            # [6,768,64] -> [768, 6*64], col = h*64+e
            maps[name] = np.ascontiguousarray(
                np.asarray(W[hs:hs + HL]).transpose(1, 0, 2)
                .reshape(D, HL * DH).astype(bf))
        in_maps.append(maps)
    return in_maps


def assemble_output(per_core_results):
    y_full = np.zeros((B, S, H * DH), np.float32)
    for core in range(NCORES):
        ya = per_core_results[core]["y_aug"]  # [6, 65, 1024]
        b, hs = core // 2, (core % 2) * HL
        ctxs = ya[:, 0:DH, :] / ya[:, DH:DH + 1, :]          # [6, 64, 1024]
        y_full[b, :, hs * DH:(hs + HL) * DH] = (
            ctxs.transpose(2, 0, 1).reshape(S, HL * DH))
    return y_full


def kernel(x, Wq, Wk, Wv):
    nc = _get_program()
    in_maps = make_in_maps(x, Wq, Wk, Wv)
    res = run_bass_kernel_spmd(nc, in_maps, core_ids=list(range(NCORES)))
    return assemble_output(res.results)


# revision 25
# speedup vs baseline: 1.0030x; 1.0030x over previous
"""Multi-head causal attention (B=4,S=1024,D=768,H=12,Dh=64) on 8 trn2 cores.

Sharding: core c handles batch b=c//2 and the 6 heads hs=(c%2)*6 .. hs+6
(head-axis tensor parallel x batch parallel; 8 cores = 4 batches x 2 head-halves).

Per-core on-chip dataflow (bf16 matmul operands, fp32 PSUM accumulation):
  xT [768,1024] (host-pretransposed bf16), W{q,k,v} stacked [768, 384] bf16
  qT/kT = W-chunk.T(lhsT) @ xT    -> [64,1024] per head (transposed layout)
  v     = xT-chunk.T @ Wv          -> [1024, 6*65] per t-chunk (65th col = ones)
  scoresT[t,s] tiles = kT-chunk(lhsT) x qT(rhs); fully-causal tiles skipped,
  diag tiles masked by accumulating identity @ (-30000 strict-lower-tri) in PSUM
  exp via ScalarE Exp(scale=1/8) straight from PSUM into a flat bf16 SBUF buffer
  ctxT_aug[65, s] = sum_j v_aug_j(lhsT) @ expT_j  (row 64 = softmax denominator)
  y_aug[h, 0:65, s] DMA'd out fp32; host divides by denominators + transposes.
"""

import threading
from contextlib import ExitStack

import ml_dtypes
import numpy as np

import concourse.bass as bass
import concourse.tile as tile
from concourse import bacc, mybir
from concourse.bass_utils import run_bass_kernel_spmd

B, S, D, H, DH = 4, 1024, 768, 12, 64
NCORES = 8
HL = H // 2          # 6 local heads per core
KC = D // 128        # 6 contraction chunks
NPAIR = HL // 2      # head pairs for qk projection
F32 = mybir.dt.float32
BF16 = mybir.dt.bfloat16
MASK_VAL = -30000.0


def _attn_groups():
    """Chunk table for one head's scoresT, packed into [128,1024] PSUM groups.

    A chunk (j, c) is the scoresT tile for t-chunk j (rows j*128..j*128+128)
    and s-range [s0, s0+w) inside output half c (s in [512c, 512c+512)).
    Only causal-relevant chunks exist. `diag` chunks need the triangular mask
    added to their first 128 columns. `ps_off` is the column offset inside the
    group's PSUM tile (each chunk stays inside one 512-col PSUM bank);
    `off` is the offset in the per-head flat exp buffer.
    """
    def chunk(j, c, ps_off):
        s0 = max(512 * c, 128 * j)
        w = 512 * (c + 1) - s0
        return dict(j=j, c=c, s0=s0, w=w, diag=(s0 == 128 * j), ps_off=ps_off)

    groups = [
        [chunk(0, 1, 0), chunk(0, 0, 512)],
        [chunk(1, 1, 0), chunk(1, 0, 512), chunk(7, 1, 896)],
        [chunk(2, 1, 0), chunk(2, 0, 512), chunk(6, 1, 768)],
        [chunk(3, 1, 0), chunk(3, 0, 512), chunk(5, 1, 640)],
        [chunk(4, 1, 0)],
    ]
    base = 0
    for g in groups:
        for ch in g:
            ch["off"] = base + ch["ps_off"]
        g_w = max(ch["ps_off"] + ch["w"] for ch in g)
        base += g_w
    total = base  # 4608
    return groups, total


def _emit_kernel(ctx: ExitStack, tc: tile.TileContext, xT, wq, wk, wv, im, y):
    nc = tc.nc
    groups, exp_cols = _attn_groups()

    # identity + causal mask arrive as a tiny host input (generating them
    # on GpSimd costs ~6us and delays the PE warm-up)
    const = ctx.enter_context(tc.tile_pool(name="const", bufs=1))
    im_sb = const.tile([128, 2, 128], BF16)
    nc.sync.dma_start(out=im_sb, in_=im[:, :, :])
    ident = im_sb[:, 0, :]
    mask = im_sb[:, 1, :]

    qk_pool = ctx.enter_context(tc.tile_pool(name="qk", bufs=1))
    qT = qk_pool.tile([128, NPAIR, S], BF16)  # partitions: (h%2)*64+e, pair, s
    kT = qk_pool.tile([128, NPAIR, S], BF16)
    v_sb = qk_pool.tile([128, 8, HL * (DH + 1)], BF16)  # [t_rel, t_chunk, h*65+x]

    # pools (PSUM budget: pj 2 banks + sg 1x4 + cx 2 = 8)
    xtw = ctx.enter_context(tc.tile_pool(name="xtw", bufs=1))
    pj = ctx.enter_context(tc.tile_pool(name="pj", bufs=1, space="PSUM"))
    sg = ctx.enter_context(tc.tile_pool(name="sg", bufs=1, space="PSUM"))
    cx = ctx.enter_context(tc.tile_pool(name="cx", bufs=2, space="PSUM"))
    ex = ctx.enter_context(tc.tile_pool(name="ex", bufs=3))
    yst = ctx.enter_context(tc.tile_pool(name="yst", bufs=3))

    # PE warm-up: ~3.5us of dummy matmuls into a scratch PSUM bank so the HAM
    # clock gate opens (K=8/8, 2.4 GHz) before the real matmuls arrive.
    warm = pj.tile([128, 128], F32, tag="pjq0", name="warm")
    for i in range(44):
        nc.tensor.matmul(out=warm, lhsT=ident, rhs=mask,
                         start=(i == 0), stop=(i == 43))

    xt = xtw.tile([128, KC, S], BF16)
    w_q = xtw.tile([128, KC, HL * DH], BF16)
    w_k = xtw.tile([128, KC, HL * DH], BF16)
    w_v = xtw.tile([128, KC, HL * DH], BF16)
    # per-chunk loads spread over four DMA queues so chunk 0 lands fast and
    # the four streams share HBM bandwidth
    for kc in range(KC):
        nc.sync.dma_start(out=xt[:, kc, :], in_=xT[kc * 128:(kc + 1) * 128, :])
        nc.scalar.dma_start(out=w_q[:, kc, :], in_=wq[kc * 128:(kc + 1) * 128, :])
        nc.scalar.dma_start(out=w_k[:, kc, :], in_=wk[kc * 128:(kc + 1) * 128, :])
        nc.gpsimd.dma_start(out=w_v[:, kc, :], in_=wv[kc * 128:(kc + 1) * 128, :])

    # ---- PE filler machinery: engines run their streams in order, so the
    # scores groups (paced by the Scalar-engine exp) must have independent
    # matmul work interleaved into the PE stream to avoid idle gaps.
    fillers = []  # list of (est_ns, emit_fn)

    def emit_fillers(budget_ns):
        while fillers and budget_ns > 0:
            est, fn = fillers.pop(0)
            fn()
            budget_ns -= est

    def proj_qk_units(pp):
        """q/k projection for pair pp as filler units (kc-outer accumulate)."""
        units = []
        for w_all, dst in ((w_q, qT), (w_k, kT)):
            pss = [pj.tile([128, 512], F32, tag=f"pjq{i}", name=f"ps{pp}{i}")
                   for i in range(2)]

            def unit(kcs, w_all=w_all, pss=pss, pp=pp, dst=dst):
                def emit():
                    for kc in kcs:
                        for i, ps in enumerate(pss):
                            nc.tensor.matmul(
                                out=ps,
                                lhsT=w_all[:, kc, pp * 128:(pp + 1) * 128],
                                rhs=xt[:, kc, i * 512:(i + 1) * 512],
                                start=(kc == 0), stop=(kc == KC - 1),
                            )
                    if kcs[-1] == KC - 1:
                        for i, ps in enumerate(pss):
                            nc.vector.tensor_copy(
                                out=dst[:, pp, i * 512:(i + 1) * 512], in_=ps)
                return emit
            units.append((900, unit([0, 1])))
            units.append((900, unit([2, 3])))
            units.append((900, unit([4, 5])))
        return units

    def proj_v_unit(j):
        def emit():
            psv = pj.tile([128, HL * DH], F32, tag=f"pjq{j % 2}", name=f"psv{j}")
            for kc in range(KC):
                nc.tensor.matmul(
                    out=psv,
                    lhsT=xt[:, kc, j * 128:(j + 1) * 128],
                    rhs=w_v[:, kc, :],
                    start=(kc == 0), stop=(kc == KC - 1),
                )
            v_dst = v_sb[:, j, :].rearrange("p (h x) -> p h x", h=HL)
            nc.vector.tensor_copy(
                out=v_dst[:, :, 0:DH],
                in_=psv.rearrange("p (h e) -> p h e", h=HL),
            )
            nc.vector.memset(v_dst[:, :, DH:DH + 1], 1.0)
        return (1100, emit)

    chunks = [ch for g in groups for ch in g]

    def ctx_unit(h, exp_pair, c):
        def emit():
            cc = sorted((ch for ch in chunks if ch["c"] == c),
                        key=lambda t: t["j"])
            pc = cx.tile([DH + 1, 512], F32, tag="cx", name=f"pc{h}{c}")
            for idx, ch in enumerate(cc):
                nc.tensor.matmul(
                    out=pc[:, ch["s0"] - 512 * c: ch["s0"] - 512 * c + ch["w"]],
                    lhsT=v_sb[:, ch["j"], :].rearrange(
                        "p (hh x) -> p hh x", hh=HL)[:, h, :],
                    rhs=exp_pair[:, h % 2, ch["off"]:ch["off"] + ch["w"]],
                    start=(idx == 0), stop=(idx == len(cc) - 1),
                )
            yt = yst.tile([DH + 1, 512], F32, tag="yst", name=f"yt{h}{c}")
            nc.vector.tensor_copy(out=yt, in_=pc)
            nc.sync.dma_start(out=y[h, :, c * 512:(c + 1) * 512], in_=yt)
        return (2200, emit)

    def scores_group(hp, g, exp_pair):
        """One scores group for both heads of pair hp into one [128,2048]
        PSUM tile (head A banks 0-1, head B banks 2-3). A/B matmuls alternate
        so their K=64 row groups (base_partition 0/64) run concurrently.
        One Exp ACT covers both heads via a strided 3D output AP."""
        g_w = max(ch["ps_off"] + ch["w"] for ch in g)
        ps = sg.tile([128, 2 * 1024], F32, tag="sg", name=f"sg{hp}")
        for bank in (0, 1):
            ops = [("mm", ch) for ch in g if ch["ps_off"] // 512 == bank]
            ops += [("mask", ch) for ch in g
                    if ch["diag"] and ch["ps_off"] // 512 == bank]
            for i, (kind, ch) in enumerate(ops):
                first, last = (i == 0), (i == len(ops) - 1)
                for a in (0, 1):
                    half = a * 64
                    off = a * 1024 + ch["ps_off"]
                    if kind == "mm":
                        nc.tensor.matmul(
                            out=ps[:, off:off + ch["w"]],
                            lhsT=kT[half:half + 64, hp,
                                    ch["j"] * 128:(ch["j"] + 1) * 128],
                            rhs=qT[half:half + 64, hp,
                                   ch["s0"]:ch["s0"] + ch["w"]],
                            start=first, stop=last,
                        )
                    else:
                        nc.tensor.matmul(
                            out=ps[:, off:off + 128],
                            lhsT=ident, rhs=mask,
                            start=first, stop=last,
                        )
        nc.scalar.activation(
            out=exp_pair[:, :, g[0]["off"]:g[0]["off"] + g_w],
            in_=ps.rearrange("p (h b) -> p h b", h=2)[:, :, 0:g_w],
            func=mybir.ActivationFunctionType.Exp,
            scale=1.0 / np.sqrt(DH),
        )

    # ---- schedule ----
    for est, fn in proj_qk_units(0):
        fn()
    fillers.extend(proj_v_unit(j) for j in range(8))

    for hp in range(NPAIR):
        # queue next pair's projections; they MUST fully emit before that
        # pair's scores groups, so they are force-drained at iteration end
        proj_next = list(proj_qk_units(hp + 1)) if hp + 1 < NPAIR else []
        fillers.extend(proj_next)
        n_proj_next = len(proj_next)

        exp_pair = ex.tile([128, 2, exp_cols], BF16, tag="exp", name=f"exp{hp}")
        for g in groups:
            scores_group(hp, g, exp_pair)
            emit_fillers(1400)

        # force-drain queued proj/v units (later stages depend on them);
        # ctx units may linger as fillers for the next pair's scores
        keep = []
        for u in fillers:
            if u in proj_next or u[0] == 1100:  # proj or v units
                u[1]()
            else:
                keep.append(u)
        fillers[:] = keep

        if hp == NPAIR - 1:
            while fillers:
                est, fn = fillers.pop(0)
                fn()
            for c in (0, 1):
                for a in (0, 1):
                    _, fn = ctx_unit(2 * hp + a, exp_pair, c)
                    fn()
        else:
            for c in (0, 1):
                for a in (0, 1):
                    fillers.append(ctx_unit(2 * hp + a, exp_pair, c))


_PROGRAM = None
_PROGRAM_LOCK = threading.Lock()


def _get_program() -> bass.Bass:
    global _PROGRAM
    with _PROGRAM_LOCK:
        if _PROGRAM is None:
            nc = bacc.Bacc(None, target_bir_lowering=False)
            xT = nc.declare_dram_parameter("xT", [D, S], BF16, isOutput=False)
            wq = nc.declare_dram_parameter("wq", [D, HL * DH], BF16, isOutput=False)
            wk = nc.declare_dram_parameter("wk", [D, HL * DH], BF16, isOutput=False)
            wv = nc.declare_dram_parameter("wv", [D, HL * DH], BF16, isOutput=False)
            im = nc.declare_dram_parameter("im", [128, 2, 128], BF16, isOutput=False)
            y = nc.declare_dram_parameter("y_aug", [HL, DH + 1, S], F32, isOutput=True)
            with tile.TileContext(nc) as tc, ExitStack() as ctx:
                _emit_kernel(ctx, tc, xT, wq, wk, wv, im, y)
            nc.finalize()  # runs Bacc passes (reg alloc, wait splitting)
            _PROGRAM = nc
    return _PROGRAM


def make_in_maps(x, Wq, Wk, Wv):
    """Per-core input dicts: batch b=core//2, heads (core%2)*6..+6."""
    bf = ml_dtypes.bfloat16
    im = np.zeros((128, 2, 128), np.float32)
    im[:, 0, :] = np.eye(128)
    t = np.arange(128)
    im[:, 1, :] = np.where(t[None, :] >= t[:, None], 0.0, MASK_VAL)
    im = im.astype(bf)
    in_maps = []
    for core in range(NCORES):
        b, hs = core // 2, (core % 2) * HL
        xTc = np.ascontiguousarray(np.asarray(x[b]).T.astype(bf))
        maps = {"xT": xTc, "im": im}
        for name, W in (("wq", Wq), ("wk", Wk), ("wv", Wv)):
            # [6,768,64] -> [768, 6*64], col = h*64+e
            maps[name] = np.ascontiguousarray(
                np.asarray(W[hs:hs + HL]).transpose(1, 0, 2)
                .reshape(D, HL * DH).astype(bf))
        in_maps.append(maps)
    return in_maps


def assemble_output(per_core_results):
    y_full = np.zeros((B, S, H * DH), np.float32)
    for core in range(NCORES):
        ya = per_core_results[core]["y_aug"]  # [6, 65, 1024]
        b, hs = core // 2, (core % 2) * HL
        ctxs = ya[:, 0:DH, :] / ya[:, DH:DH + 1, :]          # [6, 64, 1024]
        y_full[b, :, hs * DH:(hs + HL) * DH] = (
            ctxs.transpose(2, 0, 1).reshape(S, HL * DH))
    return y_full


def kernel(x, Wq, Wk, Wv):
    nc = _get_program()
    in_maps = make_in_maps(x, Wq, Wk, Wv)
    res = run_bass_kernel_spmd(nc, in_maps, core_ids=list(range(NCORES)))
    return assemble_output(res.results)


# revision 31
# speedup vs baseline: 1.0178x; 1.0147x over previous
"""Multi-head causal attention (B=4,S=1024,D=768,H=12,Dh=64) on 8 trn2 cores.

Sharding: core c handles batch b=c//2 and the 6 heads hs=(c%2)*6 .. hs+6
(head-axis tensor parallel x batch parallel; 8 cores = 4 batches x 2 head-halves).

Per-core on-chip dataflow (bf16 matmul operands, fp32 PSUM accumulation):
  xT [768,1024] (host-pretransposed bf16), W{q,k,v} stacked [768, 384] bf16
  qT/kT = W-chunk.T(lhsT) @ xT    -> [64,1024] per head (transposed layout)
  v     = xT-chunk.T @ Wv          -> [1024, 6*65] per t-chunk (65th col = ones)
  scoresT[t,s] tiles = kT-chunk(lhsT) x qT(rhs); fully-causal tiles skipped,
  diag tiles masked by accumulating identity @ (-30000 strict-lower-tri) in PSUM
  exp via ScalarE Exp(scale=1/8) straight from PSUM into a flat bf16 SBUF buffer
  ctxT_aug[65, s] = sum_j v_aug_j(lhsT) @ expT_j  (row 64 = softmax denominator)
  y_aug[h, 0:65, s] DMA'd out fp32; host divides by denominators + transposes.
"""

import threading
from contextlib import ExitStack

import ml_dtypes
import numpy as np

import concourse.bass as bass
import concourse.tile as tile
from concourse import bacc, mybir
from concourse.bass_utils import run_bass_kernel_spmd

B, S, D, H, DH = 4, 1024, 768, 12, 64
NCORES = 8
HL = H // 2          # 6 local heads per core
KC = D // 128        # 6 contraction chunks
NPAIR = HL // 2      # head pairs for qk projection
F32 = mybir.dt.float32
BF16 = mybir.dt.bfloat16
MASK_VAL = -30000.0


def _attn_groups():
    """Chunk table for one head's scoresT, packed into [128,1024] PSUM groups.

    A chunk (j, c) is the scoresT tile for t-chunk j (rows j*128..j*128+128)
    and s-range [s0, s0+w) inside output half c (s in [512c, 512c+512)).
    Only causal-relevant chunks exist. `diag` chunks need the triangular mask
    added to their first 128 columns. `ps_off` is the column offset inside the
    group's PSUM tile (each chunk stays inside one 512-col PSUM bank);
    `off` is the offset in the per-head flat exp buffer.
    """
    def chunk(j, c, ps_off):
        s0 = max(512 * c, 128 * j)
        w = 512 * (c + 1) - s0
        return dict(j=j, c=c, s0=s0, w=w, diag=(s0 == 128 * j), ps_off=ps_off)

    groups = [
        [chunk(0, 1, 0), chunk(0, 0, 512)],
        [chunk(1, 1, 0), chunk(1, 0, 512), chunk(7, 1, 896)],
        [chunk(2, 1, 0), chunk(2, 0, 512), chunk(6, 1, 768)],
        [chunk(3, 1, 0), chunk(3, 0, 512), chunk(5, 1, 640)],
        [chunk(4, 1, 0)],
    ]
    base = 0
    for g in groups:
        for ch in g:
            ch["off"] = base + ch["ps_off"]
        g_w = max(ch["ps_off"] + ch["w"] for ch in g)
        base += g_w
    total = base  # 4608
    return groups, total


def _emit_kernel(ctx: ExitStack, tc: tile.TileContext, xT, wq, wk, wv, im, y):
    nc = tc.nc
    groups, exp_cols = _attn_groups()

    # identity + 0/1 causal mask arrive as a tiny host input (generating them
    # on GpSimd costs ~6us and delays the PE warm-up)
    const = ctx.enter_context(tc.tile_pool(name="const", bufs=1))
    im_sb = const.tile([128, 2, 128], BF16)
    nc.sync.dma_start(out=im_sb, in_=im[:, :, :])
    ident = im_sb[:, 0, :]
    tri01 = im_sb[:, 1, :]  # 1 where s >= t else 0

    qk_pool = ctx.enter_context(tc.tile_pool(name="qk", bufs=1))
    qT = qk_pool.tile([128, NPAIR, S], BF16)  # partitions: (h%2)*64+e, pair, s
    kT = qk_pool.tile([128, NPAIR, S], BF16)
    v_sb = qk_pool.tile([128, 8, HL * (DH + 1)], BF16)  # [t_rel, t_chunk, h*65+x]

    # pools (PSUM budget: pj 2 banks + sg 1x4 + cx 2 = 8)
    xtw = ctx.enter_context(tc.tile_pool(name="xtw", bufs=1))
    pj = ctx.enter_context(tc.tile_pool(name="pj", bufs=1, space="PSUM"))
    sg = ctx.enter_context(tc.tile_pool(name="sg", bufs=1, space="PSUM"))
    cx = ctx.enter_context(tc.tile_pool(name="cx", bufs=2, space="PSUM"))
    ex = ctx.enter_context(tc.tile_pool(name="ex", bufs=3))
    yst = ctx.enter_context(tc.tile_pool(name="yst", bufs=3))

    # PE warm-up: ~3.5us of dummy matmuls into a scratch PSUM bank so the HAM
    # clock gate opens (K=8/8, 2.4 GHz) before the real matmuls arrive.
    warm = pj.tile([128, 128], F32, tag="pjq0", name="warm")
    for i in range(28):
        nc.tensor.matmul(out=warm, lhsT=ident, rhs=tri01,
                         start=(i == 0), stop=(i == 27))

    xt = xtw.tile([128, KC, S], BF16)
    w_q = xtw.tile([128, KC, HL * DH], BF16)
    w_k = xtw.tile([128, KC, HL * DH], BF16)
    w_v = xtw.tile([128, KC, HL * DH], BF16)
    # per-chunk loads spread over four DMA queues so chunk 0 lands fast and
    # the four streams share HBM bandwidth
    for kc in range(KC):
        nc.sync.dma_start(out=xt[:, kc, :], in_=xT[kc * 128:(kc + 1) * 128, :])
        nc.scalar.dma_start(out=w_q[:, kc, :], in_=wq[kc * 128:(kc + 1) * 128, :])
        nc.scalar.dma_start(out=w_k[:, kc, :], in_=wk[kc * 128:(kc + 1) * 128, :])
        nc.gpsimd.dma_start(out=w_v[:, kc, :], in_=wv[kc * 128:(kc + 1) * 128, :])

    # ---- PE filler machinery: engines run their streams in order, so the
    # scores groups (paced by the Scalar-engine exp) must have independent
    # matmul work interleaved into the PE stream to avoid idle gaps.
    fillers = []  # list of (est_ns, emit_fn)

    def emit_fillers(budget_ns):
        while fillers and budget_ns > 0:
            est, fn = fillers.pop(0)
            fn()
            budget_ns -= est

    def proj_qk_units(pp):
        """q/k projection for pair pp as filler units (kc-outer accumulate)."""
        units = []
        for w_all, dst in ((w_q, qT), (w_k, kT)):
            pss = [pj.tile([128, 512], F32, tag=f"pjq{i}", name=f"ps{pp}{i}")
                   for i in range(2)]

            def unit(kcs, w_all=w_all, pss=pss, pp=pp, dst=dst):
                def emit():
                    for kc in kcs:
                        for i, ps in enumerate(pss):
                            nc.tensor.matmul(
                                out=ps,
                                lhsT=w_all[:, kc, pp * 128:(pp + 1) * 128],
                                rhs=xt[:, kc, i * 512:(i + 1) * 512],
                                start=(kc == 0), stop=(kc == KC - 1),
                            )
                    if kcs[-1] == KC - 1:
                        for i, ps in enumerate(pss):
                            nc.vector.tensor_copy(
                                out=dst[:, pp, i * 512:(i + 1) * 512], in_=ps)
                return emit
            units.append((900, unit([0, 1])))
            units.append((900, unit([2, 3])))
            units.append((900, unit([4, 5])))
        return units

    def proj_v_unit(j):
        def emit():
            psv = pj.tile([128, HL * DH], F32, tag=f"pjq{j % 2}", name=f"psv{j}")
            for kc in range(KC):
                nc.tensor.matmul(
                    out=psv,
                    lhsT=xt[:, kc, j * 128:(j + 1) * 128],
                    rhs=w_v[:, kc, :],
                    start=(kc == 0), stop=(kc == KC - 1),
                )
            v_dst = v_sb[:, j, :].rearrange("p (h x) -> p h x", h=HL)
            nc.vector.tensor_copy(
                out=v_dst[:, :, 0:DH],
                in_=psv.rearrange("p (h e) -> p h e", h=HL),
            )
            nc.vector.memset(v_dst[:, :, DH:DH + 1], 1.0)
        return (1100, emit)

    chunks = [ch for g in groups for ch in g]

    def ctx_unit(h, exp_pair, c):
        def emit():
            cc = sorted((ch for ch in chunks if ch["c"] == c),
                        key=lambda t: t["j"])
            pc = cx.tile([DH + 1, 512], F32, tag="cx", name=f"pc{h}{c}")
            for idx, ch in enumerate(cc):
                nc.tensor.matmul(
                    out=pc[:, ch["s0"] - 512 * c: ch["s0"] - 512 * c + ch["w"]],
                    lhsT=v_sb[:, ch["j"], :].rearrange(
                        "p (hh x) -> p hh x", hh=HL)[:, h, :],
                    rhs=exp_pair[:, h % 2, ch["off"]:ch["off"] + ch["w"]],
                    start=(idx == 0), stop=(idx == len(cc) - 1),
                )
            yt = yst.tile([DH + 1, 512], F32, tag="yst", name=f"yt{h}{c}")
            nc.vector.tensor_copy(out=yt, in_=pc)
            nc.sync.dma_start(out=y[h, :, c * 512:(c + 1) * 512], in_=yt)
        return (2200, emit)

    def scores_group(hp, g, exp_pair):
        """One scores group for both heads of pair hp into one [128,2048]
        PSUM tile (head A banks 0-1, head B banks 2-3). A/B matmuls alternate
        so their K=64 row groups (base_partition 0/64) run concurrently.
        One Exp ACT covers both heads via a strided 3D output AP. Causal
        masking of diag chunks happens afterwards on the Vector engine
        (multiply by the 0/1 triangle), keeping the PE stream pure."""
        g_w = max(ch["ps_off"] + ch["w"] for ch in g)
        ps = sg.tile([128, 2 * 1024], F32, tag="sg", name=f"sg{hp}")
        for bank in (0, 1):
            ops = [ch for ch in g if ch["ps_off"] // 512 == bank]
            for i, ch in enumerate(ops):
                first, last = (i == 0), (i == len(ops) - 1)
                for a in (0, 1):
                    half = a * 64
                    off = a * 1024 + ch["ps_off"]
                    nc.tensor.matmul(
                        out=ps[:, off:off + ch["w"]],
                        lhsT=kT[half:half + 64, hp,
                                ch["j"] * 128:(ch["j"] + 1) * 128],
                        rhs=qT[half:half + 64, hp,
                               ch["s0"]:ch["s0"] + ch["w"]],
                        start=first, stop=last,
                    )
        nc.scalar.activation(
            out=exp_pair[:, :, g[0]["off"]:g[0]["off"] + g_w],
            in_=ps.rearrange("p (h b) -> p h b", h=2)[:, :, 0:g_w],
            func=mybir.ActivationFunctionType.Exp,
            scale=1.0 / np.sqrt(DH),
        )
        for ch in g:
            if ch["diag"]:
                for a in (0, 1):
                    sl = exp_pair[:, a, ch["off"]:ch["off"] + 128]
                    nc.vector.tensor_mul(sl, sl, tri01)

    # ---- schedule ----
    for est, fn in proj_qk_units(0):
        fn()
    fillers.extend(proj_v_unit(j) for j in range(8))

    for hp in range(NPAIR):
        # queue next pair's projections; they MUST fully emit before that
        # pair's scores groups, so they are force-drained at iteration end
        proj_next = list(proj_qk_units(hp + 1)) if hp + 1 < NPAIR else []
        fillers.extend(proj_next)
        n_proj_next = len(proj_next)

        exp_pair = ex.tile([128, 2, exp_cols], BF16, tag="exp", name=f"exp{hp}")
        for gi, g in enumerate(groups):
            scores_group(hp, g, exp_pair)
            if hp == NPAIR - 1 and gi == len(groups) - 1:
                # final group: its own ctx c0 only needs earlier groups'
                # exp, so it overlaps the last Exp ACT
                for a in (0, 1):
                    _, fn = ctx_unit(2 * hp + a, exp_pair, 0)
                    fn()
            else:
                emit_fillers(2000)

        # force-drain queued proj/v units (later stages depend on them);
        # ctx units may linger as fillers for the next pair's scores
        keep = []
        for u in fillers:
            if u in proj_next or u[0] == 1100:  # proj or v units
                u[1]()
            else:
                keep.append(u)
        fillers[:] = keep

        if hp == NPAIR - 1:
            while fillers:
                est, fn = fillers.pop(0)
                fn()
            for a in (0, 1):
                _, fn = ctx_unit(2 * hp + a, exp_pair, 1)
                fn()
        else:
            for c in (0, 1):
                for a in (0, 1):
                    fillers.append(ctx_unit(2 * hp + a, exp_pair, c))


_PROGRAM = None
_PROGRAM_LOCK = threading.Lock()


def _get_program() -> bass.Bass:
    global _PROGRAM
    with _PROGRAM_LOCK:
        if _PROGRAM is None:
            nc = bacc.Bacc(None, target_bir_lowering=False)
            xT = nc.declare_dram_parameter("xT", [D, S], BF16, isOutput=False)
            wq = nc.declare_dram_parameter("wq", [D, HL * DH], BF16, isOutput=False)
            wk = nc.declare_dram_parameter("wk", [D, HL * DH], BF16, isOutput=False)
            wv = nc.declare_dram_parameter("wv", [D, HL * DH], BF16, isOutput=False)
            im = nc.declare_dram_parameter("im", [128, 2, 128], BF16, isOutput=False)
            y = nc.declare_dram_parameter("y_aug", [HL, DH + 1, S], F32, isOutput=True)
            with tile.TileContext(nc) as tc, ExitStack() as ctx:
                _emit_kernel(ctx, tc, xT, wq, wk, wv, im, y)
            nc.finalize()  # runs Bacc passes (reg alloc, wait splitting)
            _PROGRAM = nc
    return _PROGRAM


def make_in_maps(x, Wq, Wk, Wv):
    """Per-core input dicts: batch b=core//2, heads (core%2)*6..+6."""
    bf = ml_dtypes.bfloat16
    im = np.zeros((128, 2, 128), np.float32)
    im[:, 0, :] = np.eye(128)
    t = np.arange(128)
    im[:, 1, :] = (t[None, :] >= t[:, None]).astype(np.float32)
    im = im.astype(bf)
    in_maps = []
    for core in range(NCORES):
        b, hs = core // 2, (core % 2) * HL
        xTc = np.ascontiguousarray(np.asarray(x[b]).T.astype(bf))
        maps = {"xT": xTc, "im": im}
        for name, W in (("wq", Wq), ("wk", Wk), ("wv", Wv)):
            # [6,768,64] -> [768, 6*64], col = h*64+e
            maps[name] = np.ascontiguousarray(
                np.asarray(W[hs:hs + HL]).transpose(1, 0, 2)
                .reshape(D, HL * DH).astype(bf))
        in_maps.append(maps)
    return in_maps


def assemble_output(per_core_results):
    y_full = np.zeros((B, S, H * DH), np.float32)
    for core in range(NCORES):
        ya = per_core_results[core]["y_aug"]  # [6, 65, 1024]
        b, hs = core // 2, (core % 2) * HL
        ctxs = ya[:, 0:DH, :] / ya[:, DH:DH + 1, :]          # [6, 64, 1024]
        y_full[b, :, hs * DH:(hs + HL) * DH] = (
            ctxs.transpose(2, 0, 1).reshape(S, HL * DH))
    return y_full


def kernel(x, Wq, Wk, Wv):
    nc = _get_program()
    in_maps = make_in_maps(x, Wq, Wk, Wv)
    res = run_bass_kernel_spmd(nc, in_maps, core_ids=list(range(NCORES)))
    return assemble_output(res.results)
